# revision 10
# baseline (speedup 1.0000x reference)
"""Causal attention with entity bias — Trainium2 Bass kernel, 8-core SPMD.

Problem (hardcoded): B=4, T=2048, D=1024, H=16, HD=64, fp32.
    qkv = x @ W_qkv + b_qkv ; per-head causal softmax(q k^T/sqrt(hd) + bias) v
    out = att @ W_out + b_out

Sharding: core c -> batch b = c//2, head-group hg = c%2 (8 heads, 512 dims).
Each core computes a partial out-projection (its 512 attention dims x full
W_out rows); host sums the two partials per batch and adds biases.

Device-side layout trick: everything runs "transposed" so the softmax
reduction lands on the matmul contraction axis and no on-chip transposes are
needed:
  - host passes x^T [D,T]; QKV matmuls produce q^T,k^T [dims,T] and v [T,dims]
  - S^T[k,q] = (k^T).T @ q^T  (K=64 contraction, per head)
  - softmax: exp on ACT; entity-bias+causal folded into a host-precomputed
    multiplicative tile exp(bias^T)*tril (exp(s+b) = exp(s)*exp(b); causal
    zeros); the row-sum l_q comes free from a ones-column appended to v
  - out^T[dims,q] accumulates v^T A^T via matmul over k-blocks; blocks with
    k > q are skipped entirely (causal halves the attention flops)
  - 1/l broadcast across partitions via a K=1 matmul (ones^T @ recip-row)
  - att^T is exactly the lhsT the out-projection needs; q/k biases fold into
    per-partition ACT adds; v bias and b_out fold into the host-side epilogue.
"""

import numpy as np
from contextlib import ExitStack

import concourse.bass as bass
from concourse import bacc
import concourse.tile as tile
from concourse import mybir
from concourse.bass_utils import run_bass_kernel_spmd

F32 = mybir.dt.float32
F32R = mybir.dt.float32r
EXP = mybir.ActivationFunctionType.Exp

B, T, D, H = 4, 2048, 1024, 16
HD = D // H            # 64
NCORES = 8
NH = H // 2            # 8 heads per core
DL = NH * HD           # 512 local attention dims per core
SCALE = HD ** -0.5
NKB = T // 128         # 16 key blocks
NQT = T // 512         # 4 query tiles
DK = D // 128          # 8 contraction tiles over D

LAST_RESULTS = None    # BassKernelResults of the most recent run (for test.py)


def build_program():
    nc = bacc.Bacc()
    xT = nc.declare_dram_parameter("xT", [D, T], F32R, False)
    wqk = nc.declare_dram_parameter("wqk", [DK, 8, 128, 128], F32R, False)
    wv = nc.declare_dram_parameter("wv", [D, DL], F32R, False)
    bqk = nc.declare_dram_parameter("bqk", [128, 8], F32, False)
    eb = nc.declare_dram_parameter("eb", [T, T], F32R, False)
    wout = nc.declare_dram_parameter("wout", [DL, D], F32R, False)
    ones_d = nc.declare_dram_parameter("ones_d", [128, 64], F32R, False)
    y = nc.declare_dram_parameter("y", [T, D], F32, True)

    with ExitStack() as ctx:
        tc = ctx.enter_context(tile.TileContext(nc))
        persist = ctx.enter_context(tc.tile_pool(name="persist", bufs=1))
        # qkT[0..3] = q^T tiles, qkT[4..7] = k^T tiles; tile m holds dims
        # [m*128, m*128+128) of the local 512, i.e. heads 2m,2m+1 split at
        # partition 64.
        qkT = [persist.tile([128, T], F32R, name=f"qkT{m}", tag=f"qkT{m}")
               for m in range(8)]
        # v in natural [T, dims] layout, 65-strided per head: col h*65+64 is
        # the ones column that makes the AV matmul also produce row sums.
        vsb = [persist.tile([128, NH * 65], F32R, name=f"vsb{m}", tag=f"vsb{m}")
               for m in range(NKB)]
        ones_sb = persist.tile([128, 64], F32R, name="ones_sb")
        bqk_sb = persist.tile([128, 8], F32, name="bqk_sb")

        nc.sync.dma_start(out=bqk_sb, in_=bqk[:, :])
        nc.sync.dma_start(out=ones_sb, in_=ones_d[:, :])
        for m in range(NKB):
            vv = vsb[m].rearrange("p (h x) -> p h x", x=65)
            nc.vector.tensor_copy(vv[:, :, 64], ones_sb[:, 0:NH])

        # ---------------- phase 1: QKV projections ----------------
        with tc.tile_pool(name="p1x", bufs=1) as p1x, \
             tc.tile_pool(name="p1w", bufs=1) as p1w, \
             tc.tile_pool(name="wrot", bufs=16) as wrot:
            xTs = [p1x.tile([128, T], F32R, name=f"xTs{k}", tag=f"xTs{k}")
                   for k in range(DK)]
            for k in range(DK):
                nc.sync.dma_start(out=xTs[k], in_=xT[k * 128:(k + 1) * 128, :])
            wvs = [p1w.tile([128, DL], F32R, name=f"wvs{k}", tag=f"wvs{k}")
                   for k in range(DK)]
            for k in range(DK):
                nc.sync.dma_start(out=wvs[k], in_=wv[k * 128:(k + 1) * 128, :])

            # q^T / k^T: out[dims,T]; stationary W tile reused across the 4
            # T-windows (4 concurrent PSUM accumulators).
            with tc.tile_pool(name="ps_qk", bufs=2, space="PSUM") as ps_qk:
                for m in range(8):
                    wts = []
                    for k in range(DK):
                        wt = wrot.tile([128, 128], F32R, name="wt", tag="wt")
                        nc.gpsimd.dma_start(out=wt, in_=wqk[k, m])
                        wts.append(wt)
                    pss = [ps_qk.tile([128, 512], F32, name=f"psqk{n}",
                                      tag=f"psqk{n}") for n in range(4)]
                    for k in range(DK):
                        for n in range(4):
                            nc.tensor.matmul(
                                pss[n], (wts[k]),
                                (xTs[k][:, n * 512:(n + 1) * 512]),
                                start=(k == 0), stop=(k == DK - 1))
                    for n in range(4):
                        nc.scalar.add(qkT[m][:, n * 512:(n + 1) * 512],
                                      pss[n], bqk_sb[:, m:m + 1])

            # v: out[T,dims]; lhsT = x^T tile slice (stationary), rhs = Wv.
            with tc.tile_pool(name="ps_v", bufs=4, space="PSUM") as ps_v:
                for m in range(NKB):
                    psv = ps_v.tile([128, DL], F32, name="psv", tag="psv")
                    for k in range(DK):
                        nc.tensor.matmul(
                            psv, (xTs[k][:, m * 128:(m + 1) * 128]),
                            (wvs[k]), start=(k == 0), stop=(k == DK - 1))
                    vdst = vsb[m].rearrange("p (h x) -> p h x", x=65)[:, :, 0:64]
                    nc.scalar.copy(vdst, psv.rearrange("p (h x) -> p h x", x=64))

        # ---------------- phase 2: attention + out-projection ----------------
        with tc.tile_pool(name="ebp", bufs=1) as ebp, \
             tc.tile_pool(name="sep", bufs=6) as sep, \
             tc.tile_pool(name="bcp", bufs=2) as bcp, \
             tc.tile_pool(name="wop", bufs=1) as wop, \
             tc.tile_pool(name="atp", bufs=2) as atp, \
             tc.tile_pool(name="miscp", bufs=2) as miscp, \
             tc.tile_pool(name="ysb", bufs=3) as ysbp, \
             tc.tile_pool(name="ps_s", bufs=2, space="PSUM") as ps_s, \
             tc.tile_pool(name="ps_o", bufs=1, space="PSUM") as ps_o, \
             tc.tile_pool(name="ps_bc", bufs=2, space="PSUM") as ps_bc:
            wouts = [wop.tile([128, D], F32R, name=f"wo{kd}", tag=f"wo{kd}")
                     for kd in range(4)]
            for kd in range(4):
                nc.sync.dma_start(out=wouts[kd],
                                  in_=wout[kd * 128:(kd + 1) * 128, :])
            for qt in range(NQT):
                nkb = 4 * (qt + 1)   # causal: only key blocks with k <= q
                qsl = slice(qt * 512, (qt + 1) * 512)
                ebs = []
                for kb in range(nkb):
                    ebt = ebp.tile([128, 512], F32R, name="ebt", tag=f"eb{kb}")
                    nc.gpsimd.dma_start(out=ebt,
                                      in_=eb[kb * 128:(kb + 1) * 128, qsl])
                    ebs.append(ebt)
                attT = [atp.tile([128, 512], F32R, name="attT", tag=f"attT{kd}")
                        for kd in range(4)]
                for pr in range(4):
                    # heads h0 (partitions 0:64) and h1 (64:128) of pair pr —
                    # adjacent K=64 matmuls in different PE row groups run
                    # concurrently.
                    h0, h1 = 2 * pr, 2 * pr + 1
                    o0 = ps_o.tile([65, 512], F32, name="o0", tag="o0")
                    o1 = ps_o.tile([65, 512], F32, name="o1", tag="o1")
                    for i, kb in enumerate(range(nkb)):
                        ksl = slice(kb * 128, (kb + 1) * 128)
                        s0 = ps_s.tile([128, 512], F32, name="s0", tag="s0")
                        s1 = ps_s.tile([128, 512], F32, name="s1", tag="s1")
                        nc.tensor.matmul(s0, (qkT[4 + pr][0:64, ksl]),
                                         (qkT[pr][0:64, qsl]),
                                         start=True, stop=True)
                        nc.tensor.matmul(s1, (qkT[4 + pr][64:128, ksl]),
                                         (qkT[pr][64:128, qsl]),
                                         start=True, stop=True)
                        se0 = sep.tile([128, 512], F32R, name="se0", tag="se")
                        se1 = sep.tile([128, 512], F32R, name="se1", tag="se")
                        nc.scalar.activation(se0, s0, EXP)
                        nc.scalar.activation(se1, s1, EXP)
                        nc.vector.tensor_mul(se0, se0, ebs[kb])
                        nc.vector.tensor_mul(se1, se1, ebs[kb])
                        nc.tensor.matmul(o0, (vsb[kb][:, h0 * 65:h0 * 65 + 65]),
                                         (se0), start=(kb == 0),
                                         stop=(kb == nkb - 1))
                        nc.tensor.matmul(o1, (vsb[kb][:, h1 * 65:h1 * 65 + 65]),
                                         (se1), start=(kb == 0),
                                         stop=(kb == nkb - 1))
                    for h, o in ((h0, o0), (h1, o1)):
                        linv = miscp.tile([1, 512], F32R, name="linv", tag="linv")
                        with nc.allow_low_precision(reason="f32r tag"):
                            nc.vector.reciprocal(linv, o[64:65, :])
                        bcps = ps_bc.tile([64, 512], F32, name="bcps", tag="bc")
                        nc.tensor.matmul(bcps, (ones_sb[0:1, 0:64]), (linv),
                                         start=True, stop=True)
                        bcsb = bcp.tile([64, 512], F32R, name="bcsb", tag="bcsb")
                        nc.scalar.copy(bcsb, bcps)
                        po = (h % 2) * 64
                        nc.vector.tensor_mul(attT[h // 2][po:po + 64, :],
                                             o[0:64, :], bcsb)
                # out-projection for this q window (partial: our 512 dims)
                for n in range(2):
                    for mq in range(4):
                        pp = ps_bc.tile([128, 512], F32, name="pp", tag="bc")
                        for kd in range(4):
                            nc.tensor.matmul(
                                pp, (attT[kd][:, mq * 128:(mq + 1) * 128]),
                                (wouts[kd][:, n * 512:(n + 1) * 512]),
                                start=(kd == 0), stop=(kd == 3))
                        ysb = ysbp.tile([128, 512], F32, name="ysb", tag="ysb")
                        nc.scalar.copy(ysb, pp)
                        nc.gpsimd.dma_start(
                            out=y[qt * 512 + mq * 128: qt * 512 + (mq + 1) * 128,
                                  n * 512:(n + 1) * 512],
                            in_=ysb)
    if not nc.is_finalized():
        nc.finalize()
    return nc


def make_in_maps(x, entity_bias, W_qkv, b_qkv):
    x = np.asarray(x, dtype=np.float32)
    entity_bias = np.asarray(entity_bias, dtype=np.float32)
    W_qkv = np.asarray(W_qkv, dtype=np.float32)
    b_qkv = np.asarray(b_qkv, dtype=np.float32)

    Wq, Wk, Wv = W_qkv[:, :D], W_qkv[:, D:2 * D], W_qkv[:, 2 * D:]
    bq, bk = b_qkv[:D], b_qkv[D:2 * D]

    tri = np.triu(np.ones((T, T), dtype=np.float32))  # [k,q] keep k<=q
    ebs = []
    for b in range(B):
        ebs.append((np.exp(entity_bias[b].T) * tri).astype(np.float32))

    in_maps = []
    for c in range(NCORES):
        b, hg = c // 2, c % 2
        cols = slice(hg * DL, (hg + 1) * DL)
        wqk_s = np.concatenate([Wq[:, cols] * SCALE, Wk[:, cols]], axis=1)
        wqk_t = np.ascontiguousarray(
            wqk_s.reshape(DK, 128, 8, 128).transpose(0, 2, 1, 3))
        bqk_host = np.stack(
            [bq[cols][m * 128:(m + 1) * 128] * SCALE for m in range(4)]
            + [bk[cols][m * 128:(m + 1) * 128] for m in range(4)],
            axis=1).astype(np.float32)  # [128, 8]
        in_maps.append({
            "ones_d": np.ones((128, 64), dtype=np.float32),
            "xT": np.ascontiguousarray(x[b].T),
            "wqk": wqk_t,
            "wv": np.ascontiguousarray(Wv[:, cols]),
            "bqk": bqk_host,
            "eb": ebs[b],
            "wout": None,  # filled below (needs W_out)
        })
    return in_maps


def kernel(x, entity_bias, W_qkv, b_qkv, W_out, b_out, _trace=False):
    global LAST_RESULTS
    W_out = np.asarray(W_out, dtype=np.float32)
    b_out = np.asarray(b_out, dtype=np.float32)
    b_qkv_np = np.asarray(b_qkv, dtype=np.float32)

    in_maps = make_in_maps(x, entity_bias, W_qkv, b_qkv_np)
    for c in range(NCORES):
        hg = c % 2
        in_maps[c]["wout"] = np.ascontiguousarray(
            W_out[hg * DL:(hg + 1) * DL, :])

    nc = build_program()
    res = run_bass_kernel_spmd(nc, in_maps, list(range(NCORES)), trace=_trace)
    LAST_RESULTS = res

    bv = b_qkv_np[2 * D:]
    epilogue = (b_out + bv @ W_out).astype(np.float32)
    y = np.empty((B, T, D), dtype=np.float32)
    for b in range(B):
        y[b] = res.results[2 * b]["y"] + res.results[2 * b + 1]["y"] + epilogue
    return y
